# revision 1
# baseline (speedup 1.0000x reference)
"""NeuralCDE RK4 solver as a Bass/Tile kernel on 8 Trainium2 cores.

Data-parallel over batch: B=1024 -> 128 rows per core (one partition tile).
The 127-step RK4 scan is fully unrolled; per stage:
    mm1 (PE)  : h_psum[128m,128b] = W1z.T @ zT_stage
    relu (ACT): hS = relu(h_psum + bias1(t))     (time channel folded in bias)
    mm2 (PE)  : f_psum[128b,512]  = ones.T@b2 + hS.T @ W2   (accumulated)
    tanh (ACT): fS = tanh(f_psum)
    mul  (DVE): u = fS * g(step,stage)           (g broadcast along h via AP)
    red  (DVE): k_nat[128b,64] = sum_c u
    T    (PE) : k^T accumulated into acc_psum    (RK4 weights pre-folded in g)
    stt  (DVE): z_stage_next = k^T * alpha + zT
State z^T lives in one big SBUF buffer [64, 128*128] (slot per grid point);
slots stream out to DRAM as they finish.
"""

import numpy as np
import ml_dtypes

import concourse.bacc as bacc
import concourse.bass as bass
import concourse.mybir as mybir
from concourse.tile import TileContext
from concourse.bass_utils import run_bass_kernel_spmd

F32 = mybir.dt.float32
F32R = mybir.dt.float32r
BF16 = mybir.dt.bfloat16
FP16 = mybir.dt.float16
B = 1024
L = 128
C_IN = 8
HID = 64
MLP_H = 128
INIT_H = 20
NSTEP = L - 1  # 127
NCORES = 8
BL = B // NCORES  # 128 batch rows per core

_CACHE: dict = {}


def _flags():
    import os
    return (
        os.environ.get("K_T_F32R", "0") == "1",
        os.environ.get("K_MM2_F32R", "1") == "1",
        os.environ.get("K_MUL_BF16", "1") == "1",
        os.environ.get("K_MM1_F32R", "1") == "1",
        os.environ.get("K_WARM", "0") == "1",
        os.environ.get("K_FP16_PATH", "1") == "1",
        os.environ.get("K_T_FP16", "0") == "1",
        os.environ.get("K_MM1_SPLIT", "1") == "1",
    )


def _build(nstep: int, with_b2: bool):
    import time as _time

    t_f32r, mm2_f32r, mul_bf16, mm1_f32r, warm, fp16_path, t_fp16, mm1_split = _flags()
    TD = F32R if t_f32r else F32
    if t_fp16:
        TD = FP16
    SD = F32R if mm1_f32r else F32
    MD = F32R if mm2_f32r else F32
    UD = BF16 if mul_bf16 else F32
    if fp16_path:
        MD = FP16
        UD = FP16
    t0 = _time.time()
    nc = bacc.Bacc()
    g_in = nc.dram_tensor("g", [BL, nstep * 3 * C_IN], UD, kind="ExternalInput")
    b1_in = nc.dram_tensor("bias1", [MLP_H, nstep * 3], F32, kind="ExternalInput")
    w1z_in = nc.dram_tensor("w1z", [HID, MLP_H], SD, kind="ExternalInput")
    w2_in = nc.dram_tensor("w2", [MLP_H, HID * C_IN], MD, kind="ExternalInput")
    b2_in = nc.dram_tensor("b2r", [1, HID * C_IN], MD, kind="ExternalInput")
    ones_in = nc.dram_tensor("onesr", [1, BL], MD, kind="ExternalInput")
    id_in = nc.dram_tensor("ident", [BL, BL], TD, kind="ExternalInput")
    z0t_in = nc.dram_tensor("z0t", [HID, BL], SD, kind="ExternalInput")
    w1zh_in = nc.dram_tensor("w1zh", [HID, MLP_H], FP16, kind="ExternalInput")
    zs_out = nc.dram_tensor(
        "zs", [HID, (nstep + 1) * BL], F32, kind="ExternalOutput"
    )

    NF = HID * C_IN  # 512
    with TileContext(nc) as tc:
        with (
            tc.tile_pool(name="const", bufs=1) as cp,
            tc.tile_pool(name="zst", bufs=1) as zp,
            tc.tile_pool(name="hs", bufs=3) as hp,
            tc.tile_pool(name="fs", bufs=2) as fp,
            tc.tile_pool(name="us", bufs=2) as up,
            tc.tile_pool(name="ks", bufs=3) as kp,
            tc.tile_pool(name="zc", bufs=3) as zcp,
            tc.tile_pool(name="kh", bufs=2) as khp,
            tc.tile_pool(name="ph", bufs=(4 if mm1_split else 2), space="PSUM") as ph,
            tc.tile_pool(name="pf", bufs=2, space="PSUM") as pf,
            tc.tile_pool(name="pacc", bufs=(1 if mm1_split else 2), space="PSUM") as pacc,
            tc.tile_pool(name="pks", bufs=1, space="PSUM") as pks,
            tc.tile_pool(name="pfill", bufs=1, space="PSUM") as pfill,
        ):
            gS = cp.tile([BL, nstep * 3 * C_IN], UD)
            b1S = cp.tile([MLP_H, nstep * 3], F32)
            w1zS = cp.tile([HID, MLP_H], SD)
            w1zH = cp.tile([HID, MLP_H], FP16)
            w2S = cp.tile([MLP_H, NF], MD)
            b2S = cp.tile([1, NF], MD)
            onesS = cp.tile([1, BL], MD)
            idS = cp.tile([BL, BL], TD)
            zall = zp.tile([HID, (nstep + 1) * BL], SD)
            if warm:
                wt = cp.tile([BL, BL], BF16, name="wt")
                nc.vector.memset(wt[:], 0.0)

            nc.sync.dma_start(out=gS[:], in_=g_in[:])
            nc.sync.dma_start(out=b1S[:], in_=b1_in[:])
            nc.sync.dma_start(out=w1zS[:], in_=w1z_in[:])
            nc.sync.dma_start(out=w1zH[:], in_=w1zh_in[:])
            nc.sync.dma_start(out=w2S[:], in_=w2_in[:])
            nc.sync.dma_start(out=b2S[:], in_=b2_in[:])
            nc.sync.dma_start(out=onesS[:], in_=ones_in[:])
            nc.sync.dma_start(out=idS[:], in_=id_in[:])
            nc.sync.dma_start(out=zall[:, 0:BL], in_=z0t_in[:])
            nc.sync.dma_start(out=zs_out[:, 0:BL], in_=z0t_in[:].bitcast(F32))

            if warm:
                wp = pfill.tile([BL, BL], F32, tag="fl", name="wp")
                for _ in range(48):
                    nc.tensor.matmul(
                        wp[:], lhsT=wt[:], rhs=wt[:], start=True, stop=True
                    )
            CLS = (0, 1, 1, 2)
            ALPHA = (0.5, 0.25, 0.5, 1.0 / 6.0)
            prev_accP = None
            for step in range(nstep):
                zT = zall[:, step * BL : (step + 1) * BL]
                cur = zT
                accP = None
                h_tiles = []
                if mm1_split:
                    zT_prev = zall[:, (step - 1) * BL : step * BL]
                    for s in range(4):
                        h_ps_s = ph.tile([MLP_H, BL], F32, tag="hps", name="hps")
                        has_b = not (step == 0 and s == 0)
                        nc.tensor.matmul(
                            h_ps_s[:],
                            lhsT=w1zS[:],
                            rhs=(zT_prev if (s == 0 and step > 0) else zT),
                            start=True,
                            stop=not has_b,
                        )
                        h_tiles.append(h_ps_s)
                for s in range(4):
                    col = step * 3 + CLS[s]
                    if mm1_split:
                        h_ps = h_tiles[s]
                        has_b = not (step == 0 and s == 0)
                        if has_b:
                            if s == 0:
                                ksrc, alpha_b = prev_accP, 1.0 / 6.0
                            elif s == 1:
                                ksrc, alpha_b = accP, 0.5
                            else:
                                ksrc, alpha_b = prev_ksP, 0.25 if s == 2 else 0.5
                            kh = khp.tile([HID, BL], FP16, tag="kh", name="kh")
                            nc.vector.tensor_scalar_mul(kh[:], ksrc[:], alpha_b)
                            nc.tensor.matmul(
                                h_ps[:],
                                lhsT=w1zH[:],
                                rhs=kh[:],
                                start=False,
                                stop=True,
                            )
                    else:
                        h_ps = ph.tile([MLP_H, BL], F32, tag="hps")
                        nc.tensor.matmul(
                            h_ps[:],
                            lhsT=w1zS[:],
                            rhs=cur,
                            start=True,
                            stop=True,
                        )
                    hS = hp.tile([MLP_H, BL], MD, tag="hs")
                    nc.vector.tensor_scalar(
                        hS[:],
                        h_ps[:],
                        b1S[:, col : col + 1],
                        0.0,
                        op0=mybir.AluOpType.add,
                        op1=mybir.AluOpType.max,
                    )
                    f_ps = pf.tile([BL, NF], F32, tag="fps")
                    if with_b2:
                        nc.tensor.matmul(
                            f_ps[:],
                            lhsT=onesS[:],
                            rhs=b2S[:],
                            start=True,
                            stop=False,
                        )
                    nc.tensor.matmul(
                        f_ps[:],
                        lhsT=hS[:],
                        rhs=w2S[:],
                        start=not with_b2,
                        stop=True,
                    )
                    fS = fp.tile([BL, NF], UD, tag="fs")
                    nc.scalar.activation(
                        fS[:], f_ps[:], mybir.ActivationFunctionType.Tanh
                    )
                    if warm:
                        fl1 = pfill.tile([BL, BL], F32, tag="fl", name="fl1")
                        nc.tensor.matmul(
                            fl1[:],
                            lhsT=fS[:, 0:BL],
                            rhs=fS[:, 0:BL],
                            start=True,
                            stop=True,
                        )
                    u = up.tile([BL, NF], UD, tag="u")
                    f3 = fS[:].rearrange("p (h c) -> p h c", c=C_IN)
                    u3 = u[:].rearrange("p (h c) -> p h c", c=C_IN)
                    gv = (
                        gS[:, col * C_IN : (col + 1) * C_IN]
                        .unsqueeze(1)
                        .broadcast_to((BL, HID, C_IN))
                    )
                    nc.vector.tensor_tensor(
                        out=u3, in0=f3, in1=gv, op=mybir.AluOpType.mult
                    )
                    if warm:
                        fl2 = pfill.tile([BL, BL], F32, tag="fl", name="fl2")
                        nc.tensor.matmul(
                            fl2[:],
                            lhsT=u[:, 0:BL],
                            rhs=u[:, 0:BL],
                            start=True,
                            stop=True,
                        )
                    kn = kp.tile([BL, HID], TD, tag="kn")
                    with nc.allow_low_precision("k reduce output precision"):
                        nc.vector.tensor_reduce(
                            kn[:], u3, axis=mybir.AxisListType.X, op=mybir.AluOpType.add
                        )
                    if s == 0:
                        accP = pacc.tile([HID, BL], TD, tag="acc")
                        nc.tensor.matmul(
                            accP[:],
                            lhsT=kn[:],
                            rhs=idS[:],
                            is_transpose=True,
                            start=True,
                            stop=True,
                        )
                        src = accP
                    elif s in (1, 2):
                        ksP = pks.tile([HID, BL], TD, tag="ks")
                        nc.tensor.matmul(
                            ksP[:],
                            lhsT=kn[:],
                            rhs=idS[:],
                            is_transpose=True,
                            start=True,
                            stop=True,
                        )
                        nc.tensor.matmul(
                            accP[:],
                            lhsT=kn[:],
                            rhs=idS[:],
                            is_transpose=True,
                            start=False,
                            stop=True,
                            skip_group_check=True,
                        )
                        src = ksP
                    else:
                        nc.tensor.matmul(
                            accP[:],
                            lhsT=kn[:],
                            rhs=idS[:],
                            is_transpose=True,
                            start=False,
                            stop=True,
                            skip_group_check=True,
                        )
                        src = accP
                    if s in (1, 2):
                        prev_ksP = ksP
                    if (not mm1_split) or s == 3:
                        if s < 3:
                            out_ap = zcp.tile([HID, BL], SD, tag="zc", name="zc")[:]
                        else:
                            out_ap = zall[:, (step + 1) * BL : (step + 2) * BL]
                        nc.vector.scalar_tensor_tensor(
                            out=out_ap,
                            in0=src[:],
                            scalar=ALPHA[s],
                            in1=zT,
                            op0=mybir.AluOpType.mult,
                            op1=mybir.AluOpType.add,
                        )
                        if s < 3:
                            cur = out_ap
                prev_accP = accP
                nc.sync.dma_start(
                    out=zs_out[:, (step + 1) * BL : (step + 2) * BL],
                    in_=zall[:, (step + 1) * BL : (step + 2) * BL].bitcast(F32),
                )
    import sys

    print(f"[kernel] tile trace+schedule: {_time.time()-t0:.1f}s", file=sys.stderr)
    t1 = _time.time()
    nc.finalize()
    print(f"[kernel] finalize: {_time.time()-t1:.1f}s", file=sys.stderr)
    return nc


def _get_nc(nstep: int, with_b2: bool):
    key = (nstep, with_b2) + _flags()
    if key not in _CACHE:
        _CACHE[key] = _build(nstep, with_b2)
    return _CACHE[key]


def _host_prep(coeffs, Wi1, bi1, Wi2, bi2, W1, b1, W2, b2, nstep: int):
    coeffs = np.asarray(coeffs, dtype=np.float32)
    a = coeffs[:, :, 0:8]
    b = coeffs[:, :, 8:16]
    c = coeffs[:, :, 16:24]
    d = coeffs[:, :, 24:32]

    X0 = a[:, 0]
    z0 = np.tanh(
        np.maximum(X0 @ Wi1 + bi1, 0.0).astype(np.float32) @ Wi2 + bi2
    ).astype(np.float32)

    g = np.empty((B, nstep, 3, C_IN), dtype=np.float32)
    g[:, :, 0] = b[:, :nstep]
    g[:, :, 1] = 2.0 * b[:, :nstep] + 2.0 * c[:, :nstep] + 1.5 * d[:, :nstep]
    # stage-4 derivative: dXdt at t=i+1
    last = NSTEP - 1  # 126 in full problem
    for i in range(nstep):
        if i < last:
            g[:, i, 2] = b[:, i + 1]
        else:
            g[:, i, 2] = b[:, i] + 2.0 * c[:, i] + 3.0 * d[:, i]

    tcols = np.empty((nstep, 3), dtype=np.float32)
    tcols[:, 0] = np.arange(nstep, dtype=np.float32)
    tcols[:, 1] = tcols[:, 0] + 0.5
    tcols[:, 2] = tcols[:, 0] + 1.0
    # bias1[m, step*3+cls] = b1[m] + t * W1[0, m]
    bias1 = (
        b1[None, None, :] + tcols[:, :, None] * W1[0][None, None, :]
    ).astype(np.float32)
    bias1 = bias1.reshape(nstep * 3, MLP_H).T.copy()  # [128, nstep*3]

    wdt = np.float16 if _flags()[5] else np.float32
    shared = {
        "bias1": bias1,
        "w1z": np.ascontiguousarray(W1[1:], dtype=np.float32),
        "w1zh": np.ascontiguousarray(W1[1:], dtype=np.float16),
        "w2": np.ascontiguousarray(W2, dtype=wdt),
        "b2r": np.ascontiguousarray(b2[None, :], dtype=wdt),
        "onesr": np.ones((1, BL), dtype=wdt),
        "ident": np.eye(
            BL, dtype=np.float16 if _flags()[6] else np.float32
        ),
    }
    in_maps = []
    for core in range(NCORES):
        sl = slice(core * BL, (core + 1) * BL)
        m = dict(shared)
        f = _flags()
        gdt = np.float16 if f[5] else (ml_dtypes.bfloat16 if f[2] else np.float32)
        m["g"] = np.ascontiguousarray(
            g[sl].reshape(BL, nstep * 3 * C_IN).astype(gdt)
        )
        m["z0t"] = np.ascontiguousarray(z0[sl].T)
        in_maps.append(m)
    return in_maps, z0


def kernel(coeffs, Wi1, bi1, Wi2, bi2, W1, b1, W2, b2, _nstep: int = NSTEP,
           _trace: bool = False):
    import time as _time
    import sys

    nstep = _nstep
    with_b2 = bool(np.any(np.asarray(b2)))
    nc = _get_nc(nstep, with_b2)
    in_maps, _ = _host_prep(
        coeffs, Wi1, bi1, Wi2, bi2, W1, b1, W2, b2, nstep
    )
    t0 = _time.time()
    res = run_bass_kernel_spmd(nc, in_maps, list(range(NCORES)), trace=_trace)
    print(f"[kernel] spmd run (compile+exec): {_time.time()-t0:.1f}s", file=sys.stderr)
    out = np.empty((B, nstep + 1, HID), dtype=np.float32)
    for core in range(NCORES):
        zs = res.results[core]["zs"].reshape(HID, nstep + 1, BL)
        out[core * BL : (core + 1) * BL] = zs.transpose(2, 1, 0)
    if _trace:
        kernel.last_results = res
    return out



# revision 12
# speedup vs baseline: 1.4265x; 1.4265x over previous
"""NeuralCDE RK4 solver as a Bass/Tile kernel on 8 Trainium2 cores.

Data-parallel over batch: B=1024 -> 128 rows per core. The 127-step RK4
scan is fully unrolled. Everything lives in T layout (features on
partitions, batch on the free dim); per stage the critical chain is:

    relu (ACT): hS[128m,128b] = relu(h_psum + bias1(t))      (t folded in bias)
    mm2T (PE) : f_ps[128hc,4*128b] = W2chunk_j.T @ hS        (4 MMs, one bank)
    tanh (ACT): fS = tanh(f_ps)                              (b2 == 0)
    mul  (DVE): u = fS * dxrep(step,cls)                     (dx broadcast over j)
    V    (PE) : h_psum[s+1] += (S_j@W1z*alpha).T @ u_j       (4 MMs; k never
                materialized -- feeds next stage's mm1 directly)
    S    (PE) : accP[64,128b] += S_j.T @ u_j                 (RK4 sum in PSUM)

Off the chain: one z-part matmul per stage (W1z.T @ z), one DVE op per
step for z_{i+1} = z + accP/6, and streamed DMAs for the host-built
dxrep tables (dX/dt values pre-replicated over the 16 h-groups, RK4
stage weights folded in).
"""

import numpy as np

import concourse.bacc as bacc
import concourse.bass as bass
import concourse.mybir as mybir
from concourse.tile import TileContext
from concourse.bass_utils import run_bass_kernel_spmd

F32 = mybir.dt.float32
FP16 = mybir.dt.float16
AF = mybir.ActivationFunctionType

B = 1024
L = 128
C_IN = 8
HID = 64
MLP_H = 128
NSTEP = L - 1  # 127
NCORES = 8
BL = B // NCORES  # 128 batch rows per core
NF = HID * C_IN  # 512

_CACHE: dict = {}


def _flags():
    import os

    return (
        os.environ.get("K_WARM", "1") == "1",
        int(os.environ.get("K_DXRING", "16")),
        int(os.environ.get("K_NFILL", "10")),
    )


def _build(nstep: int, with_b2: bool):
    import sys
    import time as _time

    warm, dxring, nfill = _flags()
    t0 = _time.time()
    nc = bacc.Bacc()
    dx_in = nc.dram_tensor("dxr", [128, nstep * 3 * BL], FP16, kind="ExternalInput")
    b1_in = nc.dram_tensor("bias1", [MLP_H, nstep * 3], F32, kind="ExternalInput")
    w1z_in = nc.dram_tensor("w1z", [HID, MLP_H], F32, kind="ExternalInput")
    w1zh_in = nc.dram_tensor("w1zh", [HID, MLP_H], FP16, kind="ExternalInput")
    w2_in = nc.dram_tensor("w2", [MLP_H, NF], FP16, kind="ExternalInput")
    v_in = nc.dram_tensor("vtab", [128, 2 * NF], FP16, kind="ExternalInput")
    s_in = nc.dram_tensor("spat", [128, 4 * HID], FP16, kind="ExternalInput")
    b2_in = nc.dram_tensor("b2t", [128, 4], F32, kind="ExternalInput")
    z0t_in = nc.dram_tensor("z0t", [HID, BL], F32, kind="ExternalInput")
    zs_out = nc.dram_tensor(
        "zs", [HID, (nstep + 1) * BL], F32, kind="ExternalOutput"
    )

    CLS = (0, 1, 1, 2)
    # V-table variant per stage transition s -> s+1 (alpha 0.5, 0.25, 0.5)
    VVAR = (0, 1, 0)

    with TileContext(nc) as tc:
        with (
            tc.tile_pool(name="const", bufs=1) as cp,
            tc.tile_pool(name="zst", bufs=1) as zp,
            tc.tile_pool(name="dx", bufs=dxring) as dxp,
            tc.tile_pool(name="hs", bufs=3) as hp,
            tc.tile_pool(name="fs", bufs=2) as fsp,
            tc.tile_pool(name="us", bufs=2) as up,
            tc.tile_pool(name="zh", bufs=2) as zhp,
            tc.tile_pool(name="ph", bufs=4, space="PSUM") as ph,
            tc.tile_pool(name="pf", bufs=2, space="PSUM") as pf,
            tc.tile_pool(name="pacc", bufs=1, space="PSUM") as pacc,
            tc.tile_pool(name="pfill", bufs=1, space="PSUM") as pfill,
        ):
            b1S = cp.tile([MLP_H, nstep * 3], F32)
            w1zS = cp.tile([HID, MLP_H], F32)
            w1zhS = cp.tile([HID, MLP_H], FP16)
            w2S = cp.tile([MLP_H, NF], FP16)
            vS = cp.tile([128, 2 * NF], FP16)
            sS = cp.tile([128, 4 * HID], FP16)
            b2S = cp.tile([128, 4], F32)
            zall = zp.tile([HID, (nstep + 1) * BL], F32)

            nc.sync.dma_start(out=b1S[:], in_=b1_in[:])
            nc.sync.dma_start(out=w1zS[:], in_=w1z_in[:])
            nc.sync.dma_start(out=w1zhS[:], in_=w1zh_in[:])
            nc.sync.dma_start(out=w2S[:], in_=w2_in[:])
            nc.sync.dma_start(out=vS[:], in_=v_in[:])
            nc.sync.dma_start(out=sS[:], in_=s_in[:])
            nc.sync.dma_start(out=b2S[:], in_=b2_in[:])
            nc.sync.dma_start(out=zall[:, 0:BL], in_=z0t_in[:])
            nc.sync.dma_start(out=zs_out[:, 0:BL], in_=z0t_in[:].bitcast(F32))

            if warm:
                wtL = cp.tile([128, MLP_H], FP16, name="wtL")
                wtR = cp.tile([128, NF // 2], FP16, name="wtR")
                nc.vector.memset(wtL[:], 0.0)
                nc.vector.memset(wtR[:], 0.0)
                wp = pfill.tile([128, NF], F32, tag="fl", name="wp")

            def zh_copy(z_ap):
                t = zhp.tile([HID, BL], FP16, tag="zh", name="zh")
                nc.vector.tensor_copy(t[:], z_ap)
                return t

            def h_group(znext_ap):
                """Open a step's 4 h PSUM tiles (one bank each -- start=True
                clears has_written bank-wide, so stages can't share a bank):
                z-part matmuls. Tile 0 (stage 0) is complete on its own;
                tiles 1..3 get V-matmul accumulation during stages 0..2.
                """
                tiles = []
                for s in range(4):
                    t = ph.tile([MLP_H, BL], F32, tag="hps", name=f"hps{s}")
                    nc.tensor.matmul(
                        t[:],
                        lhsT=w1zhS[:],
                        rhs=znext_ap,
                        start=True,
                        stop=(s == 0),
                        skip_group_check=True,
                    )
                    tiles.append(t)
                return tiles

            hT = h_group(zh_copy(zall[:, 0:BL])[:])

            for step in range(nstep):
                dxS = dxp.tile([128, 3 * BL], FP16, tag="dx", name="dx")
                nc.sync.dma_start(
                    out=dxS[:], in_=dx_in[:, step * 3 * BL : (step + 1) * 3 * BL]
                )
                zT = zall[:, step * BL : (step + 1) * BL]
                accP = pacc.tile([HID, BL], F32, tag="acc", name="acc")
                for s in range(4):
                    col = step * 3 + CLS[s]
                    hS = hp.tile([MLP_H, BL], FP16, tag="hs", name="hs")
                    nc.scalar.activation(
                        hS[:],
                        hT[s][:],
                        AF.Relu,
                        bias=b1S[:, col : col + 1],
                    )
                    f_ps = pf.tile([128, NF], F32, tag="fps", name="fps")
                    for j in range(4):
                        nc.tensor.matmul(
                            f_ps[:, j * BL : (j + 1) * BL],
                            lhsT=w2S[:, j * MLP_H : (j + 1) * MLP_H],
                            rhs=hS[:],
                            start=True,
                            stop=True,
                        )
                    if warm:
                        for _f in range(nfill):
                            nc.tensor.matmul(
                                wp[:, 0 : NF // 2], lhsT=wtL[:], rhs=wtR[:],
                                start=True, stop=True,
                            )
                    fS = fsp.tile([128, NF], FP16, tag="fs", name="fs")
                    if with_b2:
                        for j in range(4):
                            nc.scalar.activation(
                                fS[:, j * BL : (j + 1) * BL],
                                f_ps[:, j * BL : (j + 1) * BL],
                                AF.Tanh,
                                bias=b2S[:, j : j + 1],
                            )
                    else:
                        nc.scalar.activation(fS[:], f_ps[:], AF.Tanh)
                    u = up.tile([128, NF], FP16, tag="u", name="u")
                    u3 = u[:].rearrange("p (j b) -> p j b", j=4)
                    f3 = fS[:].rearrange("p (j b) -> p j b", j=4)
                    dxv = (
                        dxS[:, CLS[s] * BL : (CLS[s] + 1) * BL]
                        .unsqueeze(1)
                        .broadcast_to((128, 4, BL))
                    )
                    nc.vector.tensor_tensor(
                        out=u3, in0=f3, in1=dxv, op=mybir.AluOpType.mult
                    )
                    if s < 3:
                        voff = VVAR[s] * NF
                        for j in range(4):
                            nc.tensor.matmul(
                                hT[s + 1][:],
                                lhsT=vS[:, voff + j * MLP_H : voff + (j + 1) * MLP_H],
                                rhs=u[:, j * BL : (j + 1) * BL],
                                start=False,
                                stop=(j == 3),
                                skip_group_check=True,
                            )
                    for j in range(4):
                        nc.tensor.matmul(
                            accP[:],
                            lhsT=sS[:, j * HID : (j + 1) * HID],
                            rhs=u[:, j * BL : (j + 1) * BL],
                            start=(s == 0 and j == 0),
                            stop=(s == 3 and j == 3),
                            skip_group_check=True,
                        )
                znext = zall[:, (step + 1) * BL : (step + 2) * BL]
                nc.vector.scalar_tensor_tensor(
                    out=znext,
                    in0=accP[:],
                    scalar=1.0 / 6.0,
                    in1=zT,
                    op0=mybir.AluOpType.mult,
                    op1=mybir.AluOpType.add,
                )
                if step + 1 < nstep:
                    hT = h_group(zh_copy(znext)[:])
                nc.sync.dma_start(
                    out=zs_out[:, (step + 1) * BL : (step + 2) * BL],
                    in_=znext,
                )

    print(f"[kernel] tile trace+schedule: {_time.time()-t0:.1f}s", file=sys.stderr)
    t1 = _time.time()
    nc.finalize()
    print(f"[kernel] finalize: {_time.time()-t1:.1f}s", file=sys.stderr)
    return nc


def _get_nc(nstep: int, with_b2: bool):
    key = (nstep, with_b2) + _flags()
    if key not in _CACHE:
        _CACHE[key] = _build(nstep, with_b2)
    return _CACHE[key]


def _host_prep(coeffs, Wi1, bi1, Wi2, bi2, W1, b1, W2, b2, nstep: int):
    coeffs = np.asarray(coeffs, dtype=np.float32)
    a = coeffs[:, :, 0:8]
    b = coeffs[:, :, 8:16]
    c = coeffs[:, :, 16:24]
    d = coeffs[:, :, 24:32]

    X0 = a[:, 0]
    z0 = np.tanh(
        np.maximum(X0 @ Wi1 + bi1, 0.0).astype(np.float32) @ Wi2 + bi2
    ).astype(np.float32)

    # dX/dt at the three per-step sample classes, RK4 combine weights
    # folded in: class0 = dX(i) (k1 w=1), class1 = 2*dX(i+0.5) (k2+k3 w=2
    # each), class2 = dX(i+1) (k4 w=1).
    g = np.empty((B, nstep, 3, C_IN), dtype=np.float32)
    g[:, :, 0] = b[:, :nstep]
    g[:, :, 1] = 2.0 * b[:, :nstep] + 2.0 * c[:, :nstep] + 1.5 * d[:, :nstep]
    last = NSTEP - 1
    for i in range(nstep):
        if i < last:
            g[:, i, 2] = b[:, i + 1]
        else:
            g[:, i, 2] = b[:, i] + 2.0 * c[:, i] + 3.0 * d[:, i]

    tcols = np.empty((nstep, 3), dtype=np.float32)
    tcols[:, 0] = np.arange(nstep, dtype=np.float32)
    tcols[:, 1] = tcols[:, 0] + 0.5
    tcols[:, 2] = tcols[:, 0] + 1.0
    bias1 = (
        b1[None, None, :] + tcols[:, :, None] * W1[0][None, None, :]
    ).astype(np.float32)
    bias1 = bias1.reshape(nstep * 3, MLP_H).T.copy()

    w1z = np.ascontiguousarray(W1[1:], dtype=np.float32)  # (64, 128)
    # V tables: (S_j @ W1z) * alpha, laid out [p, (variant, j, m)]
    vfull = np.repeat(W1[1:], C_IN, axis=0).astype(np.float32)  # (512, 128)
    vt = np.stack([0.5 * vfull, 0.25 * vfull])  # (2, 512, 128)
    vt = vt.reshape(2, 4, 128, MLP_H).transpose(2, 0, 1, 3)
    vtab = np.ascontiguousarray(vt.reshape(128, 2 * NF), dtype=np.float16)
    # S pattern: group-of-8 partition sum, [p, (j, m)]
    q = np.arange(NF)
    sfull = (q[:, None] // C_IN == np.arange(HID)[None, :]).astype(np.float16)
    spat = np.ascontiguousarray(
        sfull.reshape(4, 128, HID).transpose(1, 0, 2).reshape(128, 4 * HID)
    )
    # b2 per-partition chunks [p, j] (only used when b2 != 0)
    b2t = np.ascontiguousarray(
        np.asarray(b2, np.float32).reshape(4, 128).T
    )

    shared = {
        "bias1": bias1,
        "w1z": w1z,
        "w1zh": np.ascontiguousarray(W1[1:], dtype=np.float16),
        "w2": np.ascontiguousarray(W2, dtype=np.float16),
        "vtab": vtab,
        "spat": spat,
        "b2t": b2t,
    }
    in_maps = []
    for core in range(NCORES):
        sl = slice(core * BL, (core + 1) * BL)
        m = dict(shared)
        arr = g[sl].astype(np.float16)  # (BL, nstep, 3, 8)
        arr = arr.transpose(3, 1, 2, 0)  # (8, nstep, 3, BL)
        arr = np.tile(arr, (16, 1, 1, 1))  # (128, nstep, 3, BL); p%8 = c
        m["dxr"] = np.ascontiguousarray(arr.reshape(128, nstep * 3 * BL))
        m["z0t"] = np.ascontiguousarray(z0[sl].T)
        in_maps.append(m)
    return in_maps, z0


def kernel(coeffs, Wi1, bi1, Wi2, bi2, W1, b1, W2, b2, _nstep: int = NSTEP,
           _trace: bool = False):
    import sys
    import time as _time

    nstep = _nstep
    with_b2 = bool(np.any(np.asarray(b2)))
    nc = _get_nc(nstep, with_b2)
    in_maps, _ = _host_prep(
        coeffs, Wi1, bi1, Wi2, bi2, W1, b1, W2, b2, nstep
    )
    t0 = _time.time()
    res = run_bass_kernel_spmd(nc, in_maps, list(range(NCORES)), trace=_trace)
    print(f"[kernel] spmd run (compile+exec): {_time.time()-t0:.1f}s", file=sys.stderr)
    out = np.empty((B, nstep + 1, HID), dtype=np.float32)
    for core in range(NCORES):
        zs = res.results[core]["zs"].reshape(HID, nstep + 1, BL)
        out[core * BL : (core + 1) * BL] = zs.transpose(2, 1, 0)
    if _trace:
        kernel.last_results = res
    return out
